# revision 1
# baseline (speedup 1.0000x reference)
"""Multi-head cross-attention (self-attention variant) on 8 Trainium2 NeuronCores.

Problem: x[1,4096,1024]; Wq/Wk/Wv[1024,1024] -> 16 heads x 64 dim; softmax(QK^T/8)V;
merge heads; @ Wo + bo -> [1,4096,1024].

Sharding: tensor-parallel over heads. Core k owns heads (2k, 2k+1) = inner cols
[128k : 128k+128]. Each core computes Q^T/K^T (in [dh, n] layout) and V for its
heads, runs flash-style attention entirely on-chip (scores never hit HBM,
softmax without max-subtraction: scores ~ N(0,1) so exp is safe in fp32), and
produces normalized head outputs O^T [128, 4096]. An AllToAll re-shards from
head-parallel to sequence-parallel: core k ends up with the full 1024-dim inner
activation for rows [512k : 512k+512], then applies the full Wo to just its row
slice. Host concatenates row slices and adds bo.

Matmuls use float32r (fp32 storage, relaxed-precision PE mode, 1 cycle/row at
N>=256 vs 4 for fp32) except the V projection (N=128, where fp32r has no
advantage).
"""
import numpy as np
from contextlib import ExitStack

N_CORES = 8
N = 4096          # sequence length
QD = 1024         # model dim
DH = 64           # head dim
HPC = 2           # heads per core
CPC = HPC * DH    # inner cols per core = 128
IC = 512          # i-chunk (query) size
NI = N // IC      # 8 chunks
JB = 128          # j-block (key) size
NJ = N // JB      # 32 blocks
SCALE = DH ** -0.5
VW = DH + 1       # V columns per head incl. ones column (65)
VBW = 2 * VW      # V block width for both heads (130)

_CACHE = {}


def _build(debug=False, repeat=1, single=False):
    from concourse import bacc, tile, mybir

    f32 = mybir.dt.float32
    fr = mybir.dt.float32r
    Exp = mybir.ActivationFunctionType.Exp

    nc = bacc.Bacc("TRN2", target_bir_lowering=False, debug=False,
                   enable_asserts=False, num_devices=1 if single else N_CORES)

    xt_d = nc.dram_tensor("xt", [QD, N], fr, kind="ExternalInput").ap()
    wq_d = nc.dram_tensor("wq", [QD, CPC], fr, kind="ExternalInput").ap()
    wk_d = nc.dram_tensor("wk", [QD, CPC], fr, kind="ExternalInput").ap()
    wv_d = nc.dram_tensor("wv", [QD, CPC], fr, kind="ExternalInput").ap()
    wo_d = nc.dram_tensor("wo", [QD, QD], fr, kind="ExternalInput").ap()
    y_d = nc.dram_tensor("y_out", [IC, QD], f32, kind="ExternalOutput").ap()
    if debug:
        qt_dbg = nc.dram_tensor("qt_dbg", [CPC, N], f32, kind="ExternalOutput").ap()
        kt_dbg = nc.dram_tensor("kt_dbg", [CPC, N], f32, kind="ExternalOutput").ap()
        v_dbg = nc.dram_tensor("v_dbg", [128, NJ * VBW], f32, kind="ExternalOutput").ap()
        a2a_dbg = nc.dram_tensor("a2a_dbg", [N_CORES * CPC, IC], f32, kind="ExternalOutput").ap()
        go_dbg = nc.dram_tensor("go_dbg", [128, 8 * IC], f32, kind="ExternalOutput").ap()

    with tile.TileContext(nc) as tc:
        with ExitStack() as ctx:
            sb = ctx.enter_context(tc.tile_pool(name="sb", bufs=1))
            xt_pool = ctx.enter_context(tc.tile_pool(name="xt", bufs=2))
            pt_pool = ctx.enter_context(tc.tile_pool(name="pt", bufs=3))
            ot_pool = ctx.enter_context(tc.tile_pool(name="ot", bufs=4))
            sm_pool = ctx.enter_context(tc.tile_pool(name="sm", bufs=4))
            y_pool = ctx.enter_context(tc.tile_pool(name="ysb", bufs=3))
            ps1 = ctx.enter_context(tc.tile_pool(name="ps1", bufs=4, space="PSUM"))
            ps2 = ctx.enter_context(tc.tile_pool(name="ps2", bufs=2, space="PSUM"))
            dram = ctx.enter_context(tc.tile_pool(name="dram", bufs=1, space="DRAM"))

            # --- static SBUF residents (per-chunk tiles so attention on
            # chunk 0 can start while later chunks are still projecting) ---
            qts = [sb.tile([CPC, IC], fr, name=f"qt{c}") for c in range(NI)]
            kts = [sb.tile([CPC, IC], fr, name=f"kt{c}") for c in range(NI)]
            vs = [sb.tile([128, 4 * VBW], fr, name=f"v{c}") for c in range(NI)]
            wq_sb = sb.tile([128, QD], fr)      # QD-tile t at cols 128t
            wk_sb = sb.tile([128, QD], fr)
            wv_sb = sb.tile([128, QD], fr)
            wo_sb = sb.tile([128, 8 * QD], fr)  # c-tile t at cols 1024t
            go_sb = sb.tile([128, 8 * IC], fr)  # gathered O^T c-tile t at cols 512t

            a2a_in = dram.tile([N_CORES * CPC, IC], fr)
            a2a_out = dram.tile([N_CORES * CPC, IC], fr)

            # weight loads
            for t in range(8):
                nc.sync.dma_start(out=wq_sb[:, 128 * t:128 * t + CPC],
                                  in_=wq_d[128 * t:128 * t + 128, :])
                nc.sync.dma_start(out=wk_sb[:, 128 * t:128 * t + CPC],
                                  in_=wk_d[128 * t:128 * t + 128, :])
                nc.sync.dma_start(out=wv_sb[:, 128 * t:128 * t + CPC],
                                  in_=wv_d[128 * t:128 * t + 128, :])
                nc.sync.dma_start(out=wo_sb[:, QD * t:QD * (t + 1)],
                                  in_=wo_d[128 * t:128 * t + 128, :])

            # ones columns of v tiles (cols 64 and 129 of each 130-wide block);
            # memset can't write float32r, so stage f32 ones and convert via DVE
            ones_sb = sb.tile([128, 4], f32)
            nc.vector.memset(ones_sb[:, :], 1.0)
            for c in range(NI):
                v3 = vs[c].rearrange("p (j w) -> p j w", w=VBW)
                nc.vector.tensor_copy(v3[:, :, DH:DH + 1], ones_sb[:, :])
                nc.vector.tensor_copy(v3[:, :, VBW - 1:VBW], ones_sb[:, :])

            for _rep in range(repeat):
                # --- phase 1: projections ---
                # One PSUM accumulation group per tile: matmul start=True clears the
                # whole bank, so groups must not share banks.
                for c in range(NI):
                    xts = []
                    for t in range(8):
                        xt_t = xt_pool.tile([128, IC], fr, name=f"xt_{t}", tag=f"xt{t}")
                        nc.sync.dma_start(
                            out=xt_t[:, :],
                            in_=xt_d[128 * t:128 * t + 128, IC * c:IC * (c + 1)])
                        xts.append(xt_t)
                    q_ps = ps1.tile([128, IC], f32, tag="ps1", name="q_ps")
                    k_ps = ps1.tile([128, IC], f32, tag="ps1", name="k_ps")
                    for t in range(8):
                        st = dict(start=(t == 0), stop=(t == 7))
                        nc.tensor.matmul(q_ps[:, :], wq_sb[:, 128 * t:128 * t + CPC],
                                         xts[t][:, :], **st)
                        nc.tensor.matmul(k_ps[:, :], wk_sb[:, 128 * t:128 * t + CPC],
                                         xts[t][:, :], **st)
                    nc.vector.tensor_copy(qts[c][:, :], q_ps[:, :])
                    nc.vector.tensor_copy(kts[c][:, :], k_ps[:, :])
                    for b in range(4):
                        v_ps = ps1.tile([128, CPC], f32, tag="ps1", name="v_ps")
                        for t in range(8):
                            nc.tensor.matmul(
                                v_ps[:, :],
                                xts[t][:, 128 * b:128 * b + 128],
                                wv_sb[:, 128 * t:128 * t + CPC],
                                start=(t == 0), stop=(t == 7))
                        for h in range(HPC):
                            nc.vector.tensor_copy(
                                vs[c][:, VBW * b + VW * h:VBW * b + VW * h + DH],
                                v_ps[:, DH * h:DH * (h + 1)])

                # --- phase 2: attention (per chunk, per head) ---
                for c in range(NI):
                    for h in range(HPC):
                        hq = qts[c][DH * h:DH * (h + 1), :]
                        acc = ps1.tile([VW, IC], f32, tag="ps1", name="acc")
                        for g in range(NJ // 2):
                            s_ps = ps2.tile([128, 2 * IC], f32, name="s_ps")
                            pt = pt_pool.tile([128, 2 * IC], fr, name="pt")
                            for u in range(2):
                                jb = 2 * g + u
                                nc.tensor.matmul(
                                    s_ps[:, IC * u:IC * (u + 1)],
                                    kts[jb // 4][DH * h:DH * (h + 1),
                                                 JB * (jb % 4):JB * (jb % 4 + 1)],
                                    hq, start=True, stop=True)
                            nc.scalar.activation(pt[:, :], s_ps[:, :], Exp, scale=SCALE)
                            for u in range(2):
                                jb = 2 * g + u
                                nc.tensor.matmul(
                                    acc[:, :],
                                    vs[jb // 4][:, VBW * (jb % 4) + VW * h:
                                                VBW * (jb % 4) + VW * (h + 1)],
                                    pt[:, IC * u:IC * (u + 1)],
                                    start=(g == 0 and u == 0),
                                    stop=(g == NJ // 2 - 1 and u == 1))
                        # normalize: rows 0..63 are head out^T, row 64 is sum(exp)
                        rsum = sm_pool.tile([1, IC], f32, name="rsum")
                        nc.vector.tensor_copy(rsum[:, :], acc[DH:DH + 1, :])
                        rcp = sm_pool.tile([1, IC], f32, name="rcp")
                        nc.vector.reciprocal(rcp[:, :], rsum[:, :])
                        rb = sm_pool.tile([DH, IC], f32, name="rb")
                        nc.gpsimd.partition_broadcast(rb[:, :], rcp[:, :])
                        ot = ot_pool.tile([DH, IC], fr, name="ot")
                        nc.vector.tensor_mul(ot[:, :], acc[0:DH, :], rb[:, :])
                        row = CPC * c + DH * h
                        nc.sync.dma_start(out=a2a_in[row:row + DH, :], in_=ot[:, :])

                # --- phase 3: reshard + output projection ---
                if single:
                    nc.sync.dma_start(out=a2a_out[:, :], in_=a2a_in[:, :])
                else:
                    nc.gpsimd.collective_compute(
                        "AllToAll", mybir.AluOpType.bypass,
                        replica_groups=[list(range(N_CORES))],
                        ins=[a2a_in.opt()], outs=[a2a_out.opt()])
                for t in range(8):
                    nc.sync.dma_start(out=go_sb[:, IC * t:IC * (t + 1)],
                                      in_=a2a_out[128 * t:128 * t + 128, :])
                if debug:
                    nc.sync.dma_start(out=a2a_dbg[:, :], in_=a2a_in[:, :].bitcast(f32))
                    nc.sync.dma_start(out=go_dbg[:, :], in_=go_sb[:, :].bitcast(f32))
                for ib in range(IC // 128):
                    for e in range(2):
                        y_ps = ps1.tile([128, 512], f32, tag="ps1", name="y_ps")
                        for t in range(8):
                            nc.tensor.matmul(
                                y_ps[:, :],
                                go_sb[:, IC * t + 128 * ib:IC * t + 128 * (ib + 1)],
                                wo_sb[:, QD * t + 512 * e:QD * t + 512 * (e + 1)],
                                start=(t == 0), stop=(t == 7))
                        y_sb = y_pool.tile([128, 512], f32, name="y_sb")
                        nc.vector.tensor_copy(y_sb[:, :], y_ps[:, :])
                        nc.sync.dma_start(
                            out=y_d[128 * ib:128 * (ib + 1), 512 * e:512 * (e + 1)],
                            in_=y_sb[:, :])
    nc.compile()
    return nc


def _get_nc():
    if "nc" not in _CACHE:
        _CACHE["nc"] = _build()
    return _CACHE["nc"]


def kernel(x, Wq, Wk, Wv, Wo, bo):
    from concourse.bass_utils import run_bass_kernel_spmd

    x = np.asarray(x, dtype=np.float32)
    Wq = np.asarray(Wq, dtype=np.float32)
    Wk = np.asarray(Wk, dtype=np.float32)
    Wv = np.asarray(Wv, dtype=np.float32)
    Wo = np.asarray(Wo, dtype=np.float32)
    bo = np.asarray(bo, dtype=np.float32)

    nc = _get_nc()
    xt = np.ascontiguousarray(x.reshape(N, QD).T)
    in_maps = []
    for k in range(N_CORES):
        cs = CPC * k
        in_maps.append({
            "xt": xt,
            "wq": np.ascontiguousarray(Wq[:, cs:cs + CPC]),
            "wk": np.ascontiguousarray(Wk[:, cs:cs + CPC]),
            "wv": np.ascontiguousarray(Wv[:, cs:cs + CPC]),
            "wo": Wo,
        })
    res = run_bass_kernel_spmd(nc, in_maps, list(range(N_CORES)))
    y = np.concatenate([res.results[k]["y_out"] for k in range(N_CORES)], axis=0)
    y = y + bo[None, :]
    return y.reshape(1, N, QD).astype(np.float32)



# revision 20
# speedup vs baseline: 1.6512x; 1.6512x over previous
"""Multi-head cross-attention (self-attention variant) on 8 Trainium2 NeuronCores.

Problem: x[1,4096,1024]; Wq/Wk/Wv[1024,1024] -> 16 heads x 64 dim; softmax(QK^T/8)V;
merge heads; @ Wo + bo -> [1,4096,1024].

Sharding: tensor-parallel over heads. Core k owns heads (2k, 2k+1) = inner cols
[128k : 128k+128]. All activations/weights are bf16 (tolerance 2e-2 vs measured
~8e-3), which keeps every matmul at 1 PE cycle/row regardless of free size.

Per core:
  - Q^T/K^T [128, 4096] projected chunk-wise (contraction = model dim, moving = x^T).
  - V projected directly in [j, d] layout (stationary = x^T tile, moving = Wv),
    one PSUM bank holds 4 j-block slots as a single accumulation group.
  - Scores S^T[j, i] per (head, j-block): stat = K^T block, mov = Q^T (512-wide).
  - exp: softmax without max-subtraction (logits ~ N(0,1)). Work is split across
    three engines to unbottleneck the Activation engine: ~20/32 tiles per chunk
    use scalar activation Exp; the rest use a Schraudolph-style fast exp2
    (bf16 bits = int16(s*A + B)) on DVE / GPSIMD via tensor_scalar + convert.
  - PV transposed: stat = P^T block [j, 128 i], mov = V [j, 64 d] -> O[i, d] in
    PSUM, 64 rows/matmul instead of 512 (2x fewer PE rows than O^T = V^T P).
    All 8 (i-block, head) slots accumulate in ONE bank as a single group;
    row-sums accumulate alongside via 1-row matmuls against a ones vector.
  - Normalize: per-partition reciprocal multiply (tensor_scalar with AP scalar).
  - O [i, d] -> O^T via DMA xbar transpose (dma_start_transpose, no PE/PSUM cost),
    AllToAll reshards head-parallel -> sequence-parallel, core k ends with rows
    [512k : 512k+512] of the merged-head activation and applies full Wo.

Emission order software-pipelines PE: scores(c) and PV(c-1) interleave at
j-group granularity so the in-order PE queue never waits on exp.
"""
import numpy as np
from contextlib import ExitStack

N_CORES = 8
N = 4096          # sequence length
QD = 1024         # model dim
DH = 64           # head dim
HPC = 2           # heads per core
CPC = HPC * DH    # inner cols per core = 128
IC = 512          # i-chunk (query) size
NI = N // IC      # 8 chunks
JB = 128          # j-block (key) size
NJ = N // JB      # 32 blocks
NG = 16           # j-groups per chunk (2 j-blocks each)
SCALE = DH ** -0.5

# fast-exp routing per chunk: r = 2*jb + h in [0, 64).
# GPSIMD cannot touch PSUM, so "mix" tiles run the scale+bias on DVE (PSUM
# reader) and the f32->i16 convert on GPSIMD from SBUF; "dve" tiles run both
# stages on DVE; everything else uses the scalar engine's Exp.
FE_DVE = frozenset({10, 30, 50})
FE_MIX = frozenset(r for r in range(64) if r % 5 in (1, 3))
# chunk 7: drain the late tiles on parallel engines so PV(7) isn't queue-gated
FE_DVE_LAST = FE_DVE | {61, 63}
FE_MIX_LAST = FE_MIX - {61, 63}
# bf16 bits of exp(s*SCALE) ~= int16(s*FE_A + FE_B):
#   FE_A = 2^23/(ln2 * 2^16) * SCALE,  FE_B = 127*128 - C/2^16 (+0.5 trunc bias)
FE_A = 184.6638356 * SCALE
FE_B = 16249.066

_CACHE = {}


def _build(single=False):
    from concourse import bacc, tile, mybir

    f32 = mybir.dt.float32
    bf16 = mybir.dt.bfloat16
    i16 = mybir.dt.int16
    Exp = mybir.ActivationFunctionType.Exp
    Mult = mybir.AluOpType.mult
    Add = mybir.AluOpType.add
    Div = mybir.AluOpType.divide

    nc = bacc.Bacc("TRN2", target_bir_lowering=False, debug=False,
                   enable_asserts=False, num_devices=1 if single else N_CORES)

    xt_d = nc.dram_tensor("xt", [QD, N], bf16, kind="ExternalInput").ap()
    wq_d = nc.dram_tensor("wq", [QD, CPC], bf16, kind="ExternalInput").ap()
    wk_d = nc.dram_tensor("wk", [QD, CPC], bf16, kind="ExternalInput").ap()
    wv_d = nc.dram_tensor("wv", [QD, CPC], bf16, kind="ExternalInput").ap()
    wo_d = nc.dram_tensor("wo", [QD, QD], bf16, kind="ExternalInput").ap()
    y_d = nc.dram_tensor("y_out", [IC, QD], f32, kind="ExternalOutput").ap()

    with tile.TileContext(nc) as tc:
        with ExitStack() as ctx:
            sb = ctx.enter_context(tc.tile_pool(name="sb", bufs=1))
            xt_pool = ctx.enter_context(tc.tile_pool(name="xt", bufs=2))
            pt_pool = ctx.enter_context(tc.tile_pool(name="pt", bufs=68))
            tmpd_pool = ctx.enter_context(tc.tile_pool(name="tmpd", bufs=3))
            tmpp_pool = ctx.enter_context(tc.tile_pool(name="tmpp", bufs=6))
            o_pool = ctx.enter_context(tc.tile_pool(name="osb", bufs=8))
            ot_pool = ctx.enter_context(tc.tile_pool(name="ot", bufs=2))
            sm_pool = ctx.enter_context(tc.tile_pool(name="sm", bufs=2))
            y_pool = ctx.enter_context(tc.tile_pool(name="ysb", bufs=8))
            sc_ps = ctx.enter_context(tc.tile_pool(name="sc", bufs=6, space="PSUM"))
            aux_ps = ctx.enter_context(tc.tile_pool(name="aux", bufs=2, space="PSUM"))
            dram = ctx.enter_context(tc.tile_pool(name="dram", bufs=1, space="DRAM"))

            # --- static SBUF residents ---
            qts = [sb.tile([CPC, IC], bf16, name=f"qt{c}") for c in range(NI)]
            kts = [sb.tile([CPC, IC], bf16, name=f"kt{c}") for c in range(NI)]
            # vs[c]: V[j, d] for j-block 4c+b at cols [128b : 128b+128]
            vs = [sb.tile([128, IC], bf16, name=f"v{c}") for c in range(NI)]
            wq_sb = sb.tile([128, 8 * CPC], bf16)   # qd-tile t at cols 128t
            wk_sb = sb.tile([128, 8 * CPC], bf16)
            wv_sb = sb.tile([128, 8 * CPC], bf16)
            wo_sb = sb.tile([128, 8 * QD], bf16)    # qd-tile t at cols 1024t
            go_sb = sb.tile([128, 8 * IC], bf16)    # a2a result, r-block at 512r
            ones_sb = sb.tile([128, 1], bf16)

            a2a_in = dram.tile([N_CORES * CPC, IC], bf16)
            a2a_out = dram.tile([N_CORES * CPC, IC], bf16)

            nc.vector.memset(ones_sb[:, :], 1.0)

            def load_w(dst, src, w):
                nc.sync.dma_start(
                    out=dst[:, :].rearrange("p (t c) -> p t c", t=8),
                    in_=src.rearrange("(t p) c -> p t c", t=8))

            def load_xt(c):
                xt_c = xt_pool.tile([128, 8 * IC], bf16, name=f"xt{c}", tag="xt")
                nc.sync.dma_start(
                    out=xt_c[:, :].rearrange("p (t i) -> p t i", t=8),
                    in_=xt_d.rearrange("(t p) n -> p t n", t=8)[:, :, IC * c:IC * (c + 1)])
                return xt_c

            def proj_qk(c, xt_c):
                q_ps = aux_ps.tile([128, IC], f32, name="q_ps", tag="aux")
                for t in range(8):
                    nc.tensor.matmul(q_ps[:, :], wq_sb[:, CPC * t:CPC * (t + 1)],
                                     xt_c[:, IC * t:IC * (t + 1)],
                                     start=(t == 0), stop=(t == 7))
                nc.vector.tensor_copy(qts[c][:, :], q_ps[:, :])
                k_ps = aux_ps.tile([128, IC], f32, name="k_ps", tag="aux")
                for t in range(8):
                    nc.tensor.matmul(k_ps[:, :], wk_sb[:, CPC * t:CPC * (t + 1)],
                                     xt_c[:, IC * t:IC * (t + 1)],
                                     start=(t == 0), stop=(t == 7))
                nc.vector.tensor_copy(kts[c][:, :], k_ps[:, :])

            def proj_v(c, xt_c):
                # V in [j, d]: stat = x^T tile (qd x j), mov = Wv tile (qd x d);
                # 4 j-block slots in one bank, single accumulation group
                v_ps = aux_ps.tile([128, IC], f32, name="v_ps", tag="aux")
                for b in range(4):
                    for t in range(8):
                        nc.tensor.matmul(
                            v_ps[:, JB * b:JB * (b + 1)],
                            xt_c[:, IC * t + JB * b:IC * t + JB * (b + 1)],
                            wv_sb[:, CPC * t:CPC * (t + 1)],
                            start=(b == 0 and t == 0), stop=(b == 3 and t == 7))
                nc.vector.tensor_copy(vs[c][:, :], v_ps[:, :])

            def scores_exp(c, jb, h):
                r = 2 * jb + h
                fe_dve = FE_DVE_LAST if c == NI - 1 else FE_DVE
                fe_mix = FE_MIX_LAST if c == NI - 1 else FE_MIX
                s_ps = sc_ps.tile([128, IC], f32, name="s_ps", tag="sc")
                nc.tensor.matmul(
                    s_ps[:, :],
                    kts[jb // 4][DH * h:DH * (h + 1),
                                 JB * (jb % 4):JB * (jb % 4 + 1)],
                    qts[c][DH * h:DH * (h + 1), :], start=True, stop=True)
                pt = pt_pool.tile([128, IC], bf16, name="pt", tag="pt")
                if r in fe_dve or r in fe_mix:
                    pool = tmpd_pool if r in fe_dve else tmpp_pool
                    tmp = pool.tile([128, IC], f32, name="fe", tag="tmp")
                    nc.vector.tensor_scalar(out=tmp[:, :], in0=s_ps[:, :],
                                            scalar1=float(FE_A),
                                            scalar2=float(FE_B),
                                            op0=Mult, op1=Add)
                    conv = nc.vector if r in fe_dve else nc.gpsimd
                    conv.tensor_copy(pt[:, :].bitcast(i16), tmp[:, :])
                else:
                    nc.scalar.activation(pt[:, :], s_ps[:, :], Exp, scale=SCALE)
                return pt

            def pv_part(jb, pts_c, acc, sums):
                # consume pt tiles of j-block jb: O[i, d] and sum rows
                g4, b = jb // 4, jb % 4
                for ib in range(4):
                    for h in range(HPC):
                        first = (jb == 0 and ib == 0 and h == 0)
                        last = (jb == NJ - 1 and ib == 3 and h == 1)
                        stat = pts_c[(h, jb)][:, JB * ib:JB * (ib + 1)]
                        s = 2 * ib + h
                        nc.tensor.matmul(
                            acc[:, DH * s:DH * (s + 1)], stat,
                            vs[g4][:, JB * b + DH * h:JB * b + DH * (h + 1)],
                            start=first, stop=last)
                        nc.tensor.matmul(
                            sums[:, s:s + 1], stat, ones_sb[:, :],
                            start=first, stop=last)

            def norm_transpose(c, acc, sums):
                # O * (1/sum(exp)) on DVE (the only vector engine allowed to
                # read PSUM): reciprocal of the 8 sums, then per-partition
                # scalar multiplies
                rcp = sm_pool.tile([128, 8], f32, name="rcp", tag="sm")
                nc.vector.reciprocal(rcp[:, :], sums[:, 0:8])
                otT = ot_pool.tile([128, IC], bf16, name="otT", tag="ot")
                for ib in range(4):
                    o_sb = o_pool.tile([128, CPC], bf16, name="osb", tag="osb")
                    for h in range(HPC):
                        s = 2 * ib + h
                        nc.vector.tensor_scalar(
                            out=o_sb[:, DH * h:DH * (h + 1)],
                            in0=acc[:, DH * s:DH * (s + 1)],
                            scalar1=rcp[:, s:s + 1], scalar2=None, op0=Mult)
                    nc.sync.dma_start_transpose(
                        out=otT[:, JB * ib:JB * (ib + 1)], in_=o_sb[:, :])
                nc.sync.dma_start(out=a2a_in[CPC * c:CPC * (c + 1), :],
                                  in_=otT[:, :])

            # --- emission ---
            load_w(wq_sb, wq_d, CPC)
            xt0 = xt_pool.tile([128, 8 * IC], bf16, name="xt0", tag="xt")
            for t in range(8):
                nc.sync.dma_start(
                    out=xt0[:, IC * t:IC * (t + 1)],
                    in_=xt_d[128 * t:128 * (t + 1), 0:IC])
            load_w(wk_sb, wk_d, CPC)
            load_w(wv_sb, wv_d, CPC)
            proj_qk(0, xt0)
            proj_v(0, xt0)

            pts = [dict() for _ in range(NI)]
            # chunk 0 scores interleaved with remaining projections; V lags
            # Q/K by two j-blocks so its PSUM slot reuse never stalls PE
            xts = {}
            for jb in range(NJ):
                if jb % 4 == 0 and jb // 4 + 1 < NI:
                    m = jb // 4 + 1
                    xts[m] = load_xt(m)
                    proj_qk(m, xts[m])
                if jb % 4 == 2 and jb // 4 + 1 < NI:
                    m = jb // 4 + 1
                    proj_v(m, xts[m])
                    del xts[m]
                for h in range(HPC):
                    pts[0][(h, jb)] = scores_exp(0, jb, h)
            load_w(wo_sb, wo_d, QD)
            # steady: scores(c) interleave with PV(c-1) at double rate in the
            # first half of each chunk, so normalize(c-1) (gpsimd) has a long
            # window before PV(c) reuses the accumulator bank. Chunk 7's own
            # PV runs in its second half, right behind its exps.
            for c in range(1, NI):
                acc = aux_ps.tile([128, IC], f32, name="acc", tag="aux")
                sums = aux_ps.tile([128, 16], f32, name="sums", tag="aux")
                for jb in range(NJ):
                    for h in range(HPC):
                        pts[c][(h, jb)] = scores_exp(c, jb, h)
                    if jb < NJ // 2:
                        pv_part(2 * jb, pts[c - 1], acc, sums)
                        pv_part(2 * jb + 1, pts[c - 1], acc, sums)
                    elif c == NI - 1:
                        if jb == NJ // 2:
                            acc7 = aux_ps.tile([128, IC], f32, name="acc",
                                               tag="aux")
                            sums7 = aux_ps.tile([128, 16], f32, name="sums",
                                                tag="aux")
                        gg = jb - NJ // 2
                        pv_part(2 * gg, pts[c], acc7, sums7)
                        pv_part(2 * gg + 1, pts[c], acc7, sums7)
                    if jb == NJ // 2 - 1:
                        norm_transpose(c - 1, acc, sums)
                        pts[c - 1] = None
            norm_transpose(NI - 1, acc7, sums7)

            # --- reshard + output projection ---
            if single:
                nc.sync.dma_start(out=a2a_out[:, :], in_=a2a_in[:, :])
            else:
                nc.gpsimd.collective_compute(
                    "AllToAll", mybir.AluOpType.bypass,
                    replica_groups=[list(range(N_CORES))],
                    ins=[a2a_in.opt()], outs=[a2a_out.opt()])
            for r in range(8):
                nc.sync.dma_start(out=go_sb[:, IC * r:IC * (r + 1)],
                                  in_=a2a_out[CPC * r:CPC * (r + 1), :])
            for ib in range(4):
                for e in range(2):
                    y_ps = sc_ps.tile([128, IC], f32, name="y_ps", tag="sc")
                    for t in range(8):
                        nc.tensor.matmul(
                            y_ps[:, :],
                            go_sb[:, IC * t + JB * ib:IC * t + JB * (ib + 1)],
                            wo_sb[:, QD * t + IC * e:QD * t + IC * (e + 1)],
                            start=(t == 0), stop=(t == 7))
                    y_sb = y_pool.tile([128, IC], f32, name="y_sb", tag="ysb")
                    nc.vector.tensor_copy(y_sb[:, :], y_ps[:, :])
                    nc.sync.dma_start(
                        out=y_d[JB * ib:JB * (ib + 1), IC * e:IC * (e + 1)],
                        in_=y_sb[:, :])
    nc.compile()
    return nc


def _get_nc():
    if "nc" not in _CACHE:
        _CACHE["nc"] = _build()
    return _CACHE["nc"]


def make_in_maps(x, Wq, Wk, Wv, Wo):
    import ml_dtypes
    bf = ml_dtypes.bfloat16
    xt = np.ascontiguousarray(x.reshape(N, QD).T.astype(bf))
    wo = np.ascontiguousarray(Wo.astype(bf))
    in_maps = []
    for k in range(N_CORES):
        cs = CPC * k
        in_maps.append({
            "xt": xt,
            "wq": np.ascontiguousarray(Wq[:, cs:cs + CPC].astype(bf)),
            "wk": np.ascontiguousarray(Wk[:, cs:cs + CPC].astype(bf)),
            "wv": np.ascontiguousarray(Wv[:, cs:cs + CPC].astype(bf)),
            "wo": wo,
        })
    return in_maps


def kernel(x, Wq, Wk, Wv, Wo, bo):
    from concourse.bass_utils import run_bass_kernel_spmd

    x = np.asarray(x, dtype=np.float32)
    Wq = np.asarray(Wq, dtype=np.float32)
    Wk = np.asarray(Wk, dtype=np.float32)
    Wv = np.asarray(Wv, dtype=np.float32)
    Wo = np.asarray(Wo, dtype=np.float32)
    bo = np.asarray(bo, dtype=np.float32)

    nc = _get_nc()
    in_maps = make_in_maps(x, Wq, Wk, Wv, Wo)
    res = run_bass_kernel_spmd(nc, in_maps, list(range(N_CORES)))
    y = np.concatenate([res.results[k]["y_out"] for k in range(N_CORES)], axis=0)
    y = y + bo[None, :]
    return y.reshape(1, N, QD).astype(np.float32)
